# revision 28
# baseline (speedup 1.0000x reference)
"""MHA with KV cache on 8 trn2 NeuronCores — tensor-parallel over heads.

Problem (hardcoded): B=2, S=1024, HIDDEN=2048, HEADS=16, HEAD_DIM=128,
PAST=1024, KV=2048.  out = softmax(mask(q k^T / sqrt(d))) v -> o_proj.

Sharding: 2 heads per core (column-parallel qkv, row-parallel o_proj);
host sums the 8 partial outputs.

v2 design (all fp16 datapath, f32 psum):
 - Projections: xt/w fp16, 16-chunk contraction into [P,512] psum, bias-add
   on DVE writing fp16 qT/kT/vT in [feature, token] layout.  V additionally
   PE-transposed into vnew [kv, d] layout.
 - Attention is computed in TRANSPOSED score orientation: for each kv chunk
   c (128 kv positions), scoresT[kv, s] = kT_chunk^T qT via one stationary
   load; exp on ACT -> expT fp16 (no row sums needed).  PV uses expT slices
   as stationary and V-chunk as moving operand, where V carries an extra
   all-ones column: psum[s, 0:128] accumulates out_mid*denom, psum[s, 128]
   accumulates the softmax denominator.  Normalize = per-partition scalar
   multiply by 1/denom on the psum->sbuf copy, then one PE transpose per
   128-query block back to [d, s] for o_proj.  No probs transposes, no
   probs normalization pass, no row-sum reductions.
 - Causal mask: per new-kv chunk only queries s >= chunk start are
   computed; one strictly-lower-triangular -1e9 add on the diagonal block.
   The 128 padded kv of batch 1 are skipped (last new chunk dropped).
 - o_proj: fp16 matmuls, bias-add folded into psum->sbuf copy alternating
   ACT/DVE, output staged 8 row-blocks at a time into fp16 DMAs.
 - Emission order interleaves phases (proj b0, attn(0,0), proj b1,
   attn(0,1), attn(1,0), oproj b0, attn(1,1), oproj b1) so ACT-bound
   attention stretches overlap PE-bound projection/o_proj stretches.
"""
import numpy as np

import concourse.bass as bass
import concourse.mybir as mybir
import concourse.tile as tile
from concourse import bacc
from concourse.bass_utils import run_bass_kernel_spmd
from concourse.masks import make_identity

FP32 = mybir.dt.float32
FP16 = mybir.dt.float16
AF = mybir.ActivationFunctionType

B, S, HID, HEADS, D, PAST = 2, 1024, 2048, 16, 128, 1024
KV = PAST + S
P = 128
NCORES = 8
HPC = HEADS // NCORES          # heads per core = 2
CD = HPC * D                   # per-core projection dims = 256
TOK = B * S                    # 2048 flattened tokens
NEG = -1e9
KC = HID // P                  # 16 contraction chunks
TCK = 512                      # token chunk for projections / o_proj
NPC = PAST // P                # past kv chunks = 8
NJ = S // P                    # query blocks per batch = 8


def build(reps=1, loop_n=None, phases=(1, 2, 3, 4)):
    nc = bacc.Bacc()

    xt = nc.dram_tensor("xt", [HID, TOK], FP16, kind="ExternalInput")
    wq = nc.dram_tensor("wq", [HID, CD], FP16, kind="ExternalInput")
    wk = nc.dram_tensor("wk", [HID, CD], FP16, kind="ExternalInput")
    wv = nc.dram_tensor("wv", [HID, CD], FP16, kind="ExternalInput")
    wo = nc.dram_tensor("wo", [CD, HID], FP16, kind="ExternalInput")
    bq = nc.dram_tensor("bq", [CD], FP32, kind="ExternalInput")
    bk = nc.dram_tensor("bk", [CD], FP32, kind="ExternalInput")
    bv = nc.dram_tensor("bv", [CD], FP32, kind="ExternalInput")
    bo = nc.dram_tensor("bo", [HID], FP32, kind="ExternalInput")
    pkt = nc.dram_tensor("pkt", [B, HPC, D, PAST], FP16, kind="ExternalInput")
    pvp = nc.dram_tensor("pvp", [B, HPC, PAST, D + 1], FP16,
                         kind="ExternalInput")
    maskl = nc.dram_tensor("maskl", [P, P], FP16, kind="ExternalInput")
    outT = nc.dram_tensor("outT", [HID, TOK], FP16, kind="ExternalOutput")

    with tile.TileContext(nc) as tc:
        with (
            tc.tile_pool(name="consts", bufs=1) as consts,
            tc.tile_pool(name="acts", bufs=1) as acts,
            tc.tile_pool(name="xtp", bufs=2) as xtp,
            tc.tile_pool(name="expp", bufs=20) as expp,
            tc.tile_pool(name="stg", bufs=2) as stgp,
            tc.tile_pool(name="small", bufs=4) as small,
            tc.tile_pool(name="ps", bufs=2, space="PSUM") as psp,
            tc.tile_pool(name="scp", bufs=2, space="PSUM") as scp,
            tc.tile_pool(name="pvq", bufs=2, space="PSUM") as pvq,
        ):
            ident = consts.tile([P, P], FP16)
            make_identity(nc, ident)
            maskl_sb = consts.tile([P, P], FP16)
            nc.scalar.dma_start(maskl_sb[:], maskl[:])
            bq_sb = consts.tile([P, HPC], FP32)
            nc.scalar.dma_start(bq_sb[:], bq.rearrange("(c p) -> p c", p=P))
            bk_sb = consts.tile([P, HPC], FP32)
            nc.scalar.dma_start(bk_sb[:], bk.rearrange("(c p) -> p c", p=P))
            bv_sb = consts.tile([P, HPC], FP32)
            nc.scalar.dma_start(bv_sb[:], bv.rearrange("(c p) -> p c", p=P))
            bo_sb = consts.tile([P, KC], FP32)
            nc.scalar.dma_start(bo_sb[:], bo.rearrange("(c p) -> p c", p=P))

            qT = acts.tile([P, HPC, TOK], FP16, tag="qT")
            kT = acts.tile([P, HPC, TOK], FP16, tag="kT")
            vT = acts.tile([P, HPC, TOK], FP16, tag="vT")
            omT = acts.tile([P, HPC, TOK], FP16, tag="omT")
            vnew = acts.tile([P, B, HPC, NJ, D + 1], FP16, tag="vnew")
            pkt_sb = acts.tile([P, B, HPC, PAST], FP16, tag="pkt")
            pv_sb = acts.tile([P, B, HPC, NPC, D + 1], FP16, tag="pv")
            wq_sb = acts.tile([P, KC, CD], FP16, tag="wq")
            wk_sb = acts.tile([P, KC, CD], FP16, tag="wk")
            wv_sb = acts.tile([P, KC, CD], FP16, tag="wv")
            wo_sb = acts.tile([P, HPC, HID], FP16, tag="wo")

            # ones column of vnew is written once; loop bodies only touch
            # columns 0:D
            nc.any.memset(vnew[:, :, :, :, D:D + 1], 1.0)

            if loop_n is not None:
                env = dict(locals())
                with tc.For_i(0, loop_n, 1):
                    _body(nc, tc, env, phases)
            else:
                for _ in range(reps):
                    _body(nc, tc, dict(locals()), phases)

    nc.finalize()
    return nc


def _body(nc, tc, env, phases=(1, 2, 3, 4)):
    ident = env["ident"]; maskl_sb = env["maskl_sb"]
    bq_sb = env["bq_sb"]; bk_sb = env["bk_sb"]; bv_sb = env["bv_sb"]
    bo_sb = env["bo_sb"]
    qT = env["qT"]; kT = env["kT"]; vT = env["vT"]; omT = env["omT"]
    vnew = env["vnew"]; pkt_sb = env["pkt_sb"]; pv_sb = env["pv_sb"]
    wq_sb = env["wq_sb"]; wk_sb = env["wk_sb"]; wv_sb = env["wv_sb"]
    wo_sb = env["wo_sb"]
    xt = env["xt"]; wq = env["wq"]; wk = env["wk"]; wv = env["wv"]
    wo = env["wo"]; outT = env["outT"]; pkt = env["pkt"]; pvp = env["pvp"]
    xtp = env["xtp"]; expp = env["expp"]; stgp = env["stgp"]
    small = env["small"]; psp = env["psp"]; scp = env["scp"]; pvq = env["pvq"]

    oproj_state = {"n": 0}

    def proj_unit(xt_t, w_sb, b_sb, dst, b, jb, t0):
        ps = psp.tile([P, TCK], FP32, tag="s")
        for kc in range(KC):
            nc.tensor.matmul(
                ps[:], w_sb[:, kc, jb * P:(jb + 1) * P],
                xt_t[:, kc, :],
                start=(kc == 0), stop=(kc == KC - 1))
        nc.vector.tensor_scalar_add(
            dst[:, jb, t0:t0 + TCK], ps[:], b_sb[:, jb:jb + 1])
        if dst is vT:
            i0 = (t0 % S) // P
            tp4 = psp.tile([P, TCK // P, P], FP16, tag="s")
            for g in range(TCK // P):
                nc.tensor.matmul(
                    tp4[:, g, :],
                    vT[:, jb, t0 + g * P: t0 + (g + 1) * P],
                    ident[:], is_transpose=True)
            nc.vector.tensor_copy(
                vnew[:, b, jb, i0:i0 + TCK // P, 0:D], tp4[:])

    def proj_chunk_units(b, ci):
        """Closures for one 512-token chunk: first issues the xt DMA, then
        the six (w, jb) units."""
        t0 = b * S + ci * TCK
        box = {}

        def dma():
            xt_t = xtp.tile([P, KC, TCK], FP16, tag="xt")
            nc.sync.dma_start(
                xt_t[:], xt[:, t0:t0 + TCK].rearrange("(c p) t -> p c t", p=P))
            box["xt"] = xt_t

        units = [dma]
        for w_sb, b_sb, dst in (
            (wq_sb, bq_sb, qT), (wk_sb, bk_sb, kT), (wv_sb, bv_sb, vT),
        ):
            for jb in range(HPC):
                units.append(
                    lambda w_sb=w_sb, b_sb=b_sb, dst=dst, jb=jb, t0=t0, b=b:
                    proj_unit(box["xt"], w_sb, b_sb, dst, b, jb, t0))
        return units

    def proj_half0():
        # split the critical first weight/xt DMAs in half so the first
        # matmuls start sooner on a cold start
        wqv = wq.rearrange("(c p) j -> p c j", p=P)
        nc.sync.dma_start(wq_sb[:, 0:KC // 2, :], wqv[:, 0:KC // 2, :])
        xtv = xt[:, 0:TCK].rearrange("(c p) t -> p c t", p=P)
        xt_t = xtp.tile([P, KC, TCK], FP16, tag="xt")
        nc.sync.dma_start(xt_t[:, 0:KC // 2, :], xtv[:, 0:KC // 2, :])
        nc.sync.dma_start(wq_sb[:, KC // 2:, :], wqv[:, KC // 2:, :])
        nc.sync.dma_start(xt_t[:, KC // 2:, :], xtv[:, KC // 2:, :])
        nc.sync.dma_start(wk_sb[:], wk.rearrange("(c p) j -> p c j", p=P))
        nc.sync.dma_start(wv_sb[:], wv.rearrange("(c p) j -> p c j", p=P))
        proj_unit(xt_t, wq_sb, bq_sb, qT, 0, 0, 0)
        nc.sync.dma_start(pkt_sb[:], pkt.rearrange("b h p kv -> p b h kv"))
        nc.sync.dma_start(
            pv_sb[:], pvp.rearrange("b h (c p) e -> p b h c e", p=P))
        proj_unit(xt_t, wq_sb, bq_sb, qT, 0, 1, 0)
        for w_sb, b_sb, dst in ((wk_sb, bk_sb, kT), (wv_sb, bv_sb, vT)):
            for jb in range(HPC):
                proj_unit(xt_t, w_sb, b_sb, dst, 0, jb, 0)
        units = proj_chunk_units(0, 1)
        units[0]()  # xt DMA for second chunk
        nc.sync.dma_start(wo_sb[:], wo.rearrange("(c p) m -> p c m", p=P))
        for u in units[1:]:
            u()

    def oproj_group_units(b, ci, half, tail=False):
        """Closures for one 8-row-block output group: 8 matmul+copy units,
        then the output DMA.  In the un-woven tail the scores psum pool is
        idle, so alternate psum slots with it for a deeper pipeline."""
        t0 = b * S + ci * TCK
        box = {}

        def unit(mi):
            if mi == 0:
                box["stg"] = stgp.tile([P, KC // 2, TCK], FP16, tag="stg",
                                       name="stg")
            stg = box["stg"]
            mb = half * (KC // 2) + mi
            if tail and mi % 2 == 1:
                ps = scp.tile([P, TCK], FP32, tag="sc", name="ps")
            else:
                ps = psp.tile([P, TCK], FP32, tag="s", name="ps")
            for jc in range(HPC):
                nc.tensor.matmul(
                    ps[:], wo_sb[:, jc, mb * P:(mb + 1) * P],
                    omT[:, jc, t0:t0 + TCK],
                    start=(jc == 0), stop=(jc == HPC - 1))
            if oproj_state["n"] % 2 == 0:
                nc.scalar.activation(stg[:, mi, :], ps[:], AF.Identity,
                                     bias=bo_sb[:, mb:mb + 1])
            else:
                nc.vector.tensor_scalar_add(stg[:, mi, :], ps[:],
                                            bo_sb[:, mb:mb + 1])
            oproj_state["n"] += 1

        def dma():
            nc.sync.dma_start(
                outT.rearrange("(c p) t -> p c t", p=P)
                    [:, half * (KC // 2):(half + 1) * (KC // 2),
                     t0:t0 + TCK],
                box["stg"][:])

        return [lambda mi=mi: unit(mi) for mi in range(KC // 2)] + [dma]

    N_SLOTS = NPC + 1 + NJ  # 17 emission slots per attention pair

    def attn_pair(b, h, fillers=None):
        nchunks = NPC + NJ - (1 if b == 1 else 0)
        exps = [None] * (NPC + NJ)

        def scores_chunk(c):
            if c < NPC:
                stat = pkt_sb[:, b, h, c * P:(c + 1) * P]
                s_lo = 0
            else:
                cp = c - NPC
                stat = kT[:, h, b * S + cp * P: b * S + (cp + 1) * P]
                s_lo = cp * P
            w = S - s_lo
            sc = scp.tile([P, S], FP32, tag="sc")
            for o in range(0, w, TCK):
                ww = min(TCK, w - o)
                nc.tensor.matmul(
                    sc[:, o:o + ww], stat,
                    qT[:, h, b * S + s_lo + o: b * S + s_lo + o + ww],
                    start=True, stop=True)
            ex = expp.tile([P, S], FP16, tag="exp")
            nc.scalar.activation(ex[:, 0:w], sc[:, 0:w], AF.Exp)
            if c >= NPC:
                # zero the disallowed (strictly-lower) triangle of the
                # diagonal block; the PV ones-column then sees the zeros,
                # so the softmax denominator is automatically correct
                nc.vector.tensor_mul(ex[:, 0:P], ex[:, 0:P], maskl_sb[:])
            exps[c] = (ex, s_lo)

        def pv_j(j):
            cs = [c for c in range(nchunks) if c < NPC or (c - NPC) <= j]
            pvt = pvq.tile([P, D + 1], FP32, tag="pv")
            for idx, c in enumerate(cs):
                ex, s_lo = exps[c]
                off = j * P - s_lo
                if c < NPC:
                    mov = pv_sb[:, b, h, c, :]
                else:
                    mov = vnew[:, b, h, c - NPC, :]
                nc.tensor.matmul(pvt[:], ex[:, off:off + P], mov,
                                 start=(idx == 0), stop=(idx == len(cs) - 1))
            recip = small.tile([P, 1], FP32, tag="rcp")
            nc.vector.reciprocal(recip[:], pvt[:, D:D + 1])
            osd = small.tile([P, P], FP16, tag="osd")
            nc.vector.tensor_scalar_mul(osd[:], pvt[:, 0:D], recip[:])
            tp = psp.tile([P, P], FP16, tag="s")
            nc.tensor.matmul(tp[:], osd[:], ident[:], is_transpose=True)
            t_lo = b * S + j * P
            nc.vector.tensor_copy(omT[:, h, t_lo:t_lo + P], tp[:])

        pending = list(fillers or [])

        def drain(slot):
            while pending and pending[0][0] <= slot:
                elig = sum(1 for ms, _ in pending if ms <= slot)
                budget = -(-elig // max(N_SLOTS - slot, 1))
                if budget == 0:
                    break
                for _ in range(budget):
                    if pending and pending[0][0] <= slot:
                        pending.pop(0)[1]()
                break

        for c in range(NPC + 1):
            scores_chunk(c)
            drain(c)
        for j in range(NJ):
            c = NPC + 1 + j
            if c < nchunks:
                scores_chunk(c)
            pv_j(j)
            drain(NPC + 1 + j)
        for _, f in pending:
            f()

    def ogu(b, ci, half, tail=False):
        if 3 not in phases:
            return []
        return oproj_group_units(b, ci, half, tail=tail)

    if 2 in phases:
        proj_half0()
        # proj of batch 1 woven into the two b0 attention pairs; o_proj
        # groups woven in as soon as their omT token range is complete.
        # xt DMAs are issued ahead of each pair so woven units never wait.
        p10 = proj_chunk_units(1, 0)
        p10[0]()
        attn_pair(0, 0, [(0, u) for u in p10[1:]])
        p11 = proj_chunk_units(1, 1)
        p11[0]()
        op = [(0, u) for u in p11[1:]]
        op += [(13, u) for u in ogu(0, 0, 0)]
        attn_pair(0, 1, op)
        op = [(0, u) for u in ogu(0, 0, 1)]
        op += [(5, u) for u in ogu(0, 1, 0)]
        attn_pair(1, 0, op)
        op = [(0, u) for u in ogu(0, 1, 1)]
        op += [(13, u) for u in ogu(1, 0, 0)]
        op += [(15, u) for u in ogu(1, 0, 1)]
        attn_pair(1, 1, op)
        for u in ogu(1, 1, 0, tail=True) + ogu(1, 1, 1, tail=True):
            u()
    else:
        proj_half0()
        for u in proj_chunk_units(1, 0) + proj_chunk_units(1, 1):
            u()


_cached_nc = None


def _get_nc():
    global _cached_nc
    if _cached_nc is None:
        _cached_nc = build()
    return _cached_nc


def _prep_in_maps(inputs):
    X = np.asarray(inputs["X"], dtype=np.float32)
    past_k = np.asarray(inputs["past_k"], dtype=np.float32)
    past_v = np.asarray(inputs["past_v"], dtype=np.float32)
    Wq = np.asarray(inputs["Wq"], dtype=np.float32)
    Wk = np.asarray(inputs["Wk"], dtype=np.float32)
    Wv = np.asarray(inputs["Wv"], dtype=np.float32)
    Wo = np.asarray(inputs["Wo"], dtype=np.float32)
    bq = np.asarray(inputs["bq"], dtype=np.float32)
    bk = np.asarray(inputs["bk"], dtype=np.float32)
    bv = np.asarray(inputs["bv"], dtype=np.float32)
    bo = np.asarray(inputs["bo"], dtype=np.float32)

    scale = np.float32(1.0 / np.sqrt(D))
    xt = np.ascontiguousarray(X.reshape(TOK, HID).T).astype(np.float16)
    # 1.0 where kv row <= query col (allowed), 0.0 in the strictly lower
    # triangle (kv position ahead of query)
    maskl = np.triu(np.ones((P, P), dtype=np.float16))

    pvp_full = np.concatenate(
        [past_v, np.ones((B, HEADS, PAST, 1), dtype=np.float32)],
        axis=-1).astype(np.float16)

    in_maps = []
    for c in range(NCORES):
        lo, hi = c * CD, (c + 1) * CD
        in_maps.append({
            "xt": xt,
            "wq": np.ascontiguousarray((Wq[lo:hi] * scale).T).astype(np.float16),
            "wk": np.ascontiguousarray(Wk[lo:hi].T).astype(np.float16),
            "wv": np.ascontiguousarray(Wv[lo:hi].T).astype(np.float16),
            "wo": np.ascontiguousarray(Wo[:, lo:hi].T).astype(np.float16),
            "bq": np.ascontiguousarray(bq[lo:hi] * scale),
            "bk": np.ascontiguousarray(bk[lo:hi]),
            "bv": np.ascontiguousarray(bv[lo:hi]),
            "bo": bo if c == 0 else np.zeros_like(bo),
            "pkt": np.ascontiguousarray(
                past_k[:, c * HPC:(c + 1) * HPC].transpose(0, 1, 3, 2)
            ).astype(np.float16),
            "pvp": np.ascontiguousarray(pvp_full[:, c * HPC:(c + 1) * HPC]),
            "maskl": maskl,
        })
    return in_maps


def _run(inputs, trace=False, nc=None):
    if nc is None:
        nc = _get_nc()
    in_maps = _prep_in_maps(inputs)
    res = run_bass_kernel_spmd(nc, in_maps, core_ids=list(range(NCORES)),
                               trace=trace)
    outT = res.results[0]["outT"].astype(np.float64)
    for c in range(1, NCORES):
        outT += res.results[c]["outT"]
    out = outT.T.reshape(B, S, HID).astype(np.float32)
    return out, res


def kernel(**inputs):
    out, _ = _run(inputs, trace=False)
    return out


def kernel_traced(**inputs):
    try:
        return _run(inputs, trace=True)
    except Exception:
        return _run(inputs, trace=False)
